# revision 53
# baseline (speedup 1.0000x reference)
"""Correlation cost volume kernel for Trainium2 (8 NeuronCores, data-parallel over batch).

cost[b, i, h, x] = mean_c left[b,c,h,x] * right[b,c,h,x-i],  i in [0,48), zero for x < i.

Per core (one batch element), all data bf16 (host converts; left pre-scaled 1/C):
  For each group of HG h rows:
    l_t [C, HG*320], r_t [C, HG*368] (47 left-pad zeros + data + 1 zero col).
    Per h row, 3 matmuls G[a, j] = sum_c l[c, X0+a] r_pad[c, X0+j] into psum
    bank slots (A[128x176] B[128x176] C[64x112] packed per 512-col fp32 bank,
    2 h rows per [128,1024] tile).
    Engine eviction (DVE/Act alternating, one [128, 2x464] op per 2h) writes
    the h-INTERLEAVED rect: rect[a, HG*(ci*176+j)+h] = G[a,h,ci,j], so each
    (a, ci) band row [j = a..a+48) x h is one contiguous 48*HG-elem run.
    Shear: direct SBUF->SBUF DMA. The DMA lowering only honors non-pitch
    partition strides for APs that start at partition 0 with <=32 partitions,
    so the diagonal is built per 32-partition block q:
      q=0: src-side diagonal (stride pitch+HG) -> exact 48*HG runs.
      q=1..3: plain 80-j src window (any base legal) + dst-side diagonal
      (stride pitch-HG, base 0) -> each strip holds the diagonal at col
      31*HG with windowed garbage around it (never read).
    band32 [32, 14848]: 3 exact strips (q0) + 7 windowed strips.
    PE transposes per (2h, parity e, strip): in [32, 48k] -> out [48, 32]
    at partition base e*64 (PSUM matmul outputs must start at partition
    0/32/64) into psum bt [112, 320], cols = x. Engine copy -> outg
    [112, 8*320]; two strided DMAs per group write the (k, h, x) output
    with full 320-elem (640B) runs (disparity reversed: device row
    k = cost[47-k]; host flips).
  Queue discipline: shear DMAs ride the Pool SWDGE path (non-blocking
  dispatch); SP gets l-load/out-store; Act gets only the right-load so its
  SEQ never stalls eviction/copy dispatches.
"""
import numpy as np
import ml_dtypes

import concourse.bacc as bacc
import concourse.mybir as mybir
import concourse.tile as tile
from concourse.ap import AP
from concourse.bass_utils import run_bass_kernel_spmd

B, C, H, W = 8, 128, 96, 320
D = 48
HG = 16         # h rows per group
NG = H // HG    # groups
RPAD = W + D    # 368: 47 left zeros, W data, 1 right zero
CHUNKS = [(0, 128, 0), (128, 128, 176), (256, 64, 352)]  # (X0, M, gcol)
SLOT = 464      # rect cols per h row (176+176+112)
HW = H * W
RUN = 48 * HG   # exact diagonal run per (a, ci)
WIN = 80        # windowed j span per 32-block
WSTRIP = 112 * HG   # windowed strip width (31 lead + 80 window + 1 pad)
DOFF = 31 * HG      # diagonal offset inside a windowed strip

# strips in band32 column order: (q, ci, colbase, diag_off)
_STRIPS = []
_col = 0
for _ci in range(3):
    _STRIPS.append((0, _ci, _col, 0))
    _col += RUN
for _q in range(1, 4):
    for _ci in range(3 if _q == 1 else 2):
        _STRIPS.append((_q, _ci, _col, DOFF))
        _col += WSTRIP
B32COLS = _col  # 14848

SHEAR_MODE = "sbuf"  # kept for test.py compat
_cache = {}


def _build(_mode="sbuf"):
    nc = bacc.Bacc("TRN2", target_bir_lowering=False, debug=False, num_devices=8)
    left = nc.dram_tensor("left", [C, HW], mybir.dt.bfloat16, kind="ExternalInput").ap()
    right = nc.dram_tensor("right", [C, HW], mybir.dt.bfloat16, kind="ExternalInput").ap()
    ident_in = nc.dram_tensor("ident", [128, 128], mybir.dt.bfloat16, kind="ExternalInput").ap()
    out = nc.dram_tensor("out", [D, HW], mybir.dt.bfloat16, kind="ExternalOutput").ap()

    with tile.TileContext(nc) as tc:
        with (
            tc.tile_pool(name="io", bufs=3) as io_pool,
            tc.tile_pool(name="rectp", bufs=3) as rect_pool,
            tc.tile_pool(name="bandp", bufs=2) as band_pool,
            tc.tile_pool(name="outp", bufs=4) as outg_pool,
            tc.tile_pool(name="const", bufs=1) as const_pool,
            tc.tile_pool(name="gps", bufs=3, space="PSUM") as g_pool,
            tc.tile_pool(name="bts", bufs=2, space="PSUM") as bt_pool,
        ):
            ident = const_pool.tile([128, 128], mybir.dt.bfloat16)

            def load_group(g):
                h0 = g * HG
                l_t = io_pool.tile([C, HG * W], mybir.dt.bfloat16, tag="lt")
                r_t = io_pool.tile([C, HG * RPAD], mybir.dt.bfloat16, tag="rt")
                rtp = r_t.ap[0][0]
                # pads: cols [0:47] and col 367 of each h row. Buffers rotate
                # with period 3 and loads only write data cols, so pads stay
                # zero after the first rotation.
                if g < 3:
                    nc.gpsimd.memset(
                        AP(r_t.tensor, r_t.offset, [[rtp, C], [RPAD, HG], [1, D - 1]]), 0.0)
                    nc.gpsimd.memset(
                        AP(r_t.tensor, r_t.offset + RPAD - 1, [[rtp, C], [RPAD, HG], [1, 1]]), 0.0)
                nc.sync.dma_start(out=l_t[:, :], in_=left[:, h0 * W : (h0 + HG) * W])
                r_dst = AP(r_t.tensor, r_t.offset + (D - 1), [[rtp, C], [RPAD, HG], [1, W]])
                nc.scalar.dma_start(out=r_dst, in_=right[:, h0 * W : (h0 + HG) * W])
                return l_t, r_t

            def front_unit(g, p, l_t, r_t, rect):
                rp = rect.ap[0][0]
                gt = g_pool.tile([128, 1024], mybir.dt.float32, tag="g")
                gp = gt.ap[0][0]
                for e in range(2):
                    hl = 2 * p + e
                    for X0, M, gcol in CHUNKS:
                        NW = M + D
                        nc.tensor.matmul(
                            gt[:M, 512 * e + gcol : 512 * e + gcol + NW],
                            l_t[:, hl * W + X0 : hl * W + X0 + M],
                            r_t[:, hl * RPAD + X0 : hl * RPAD + X0 + NW],
                            start=True, stop=True,
                        )
                eng = nc.vector if p % 2 == 0 else nc.scalar
                ev = eng.tensor_copy if p % 2 == 0 else eng.copy
                ev(
                    AP(rect.tensor, rect.offset + 2 * p,
                       [[rp, 128], [1, 2], [HG, SLOT]]),
                    AP(gt.tensor, gt.offset, [[gp, 128], [512, 2], [1, SLOT]]),
                )

            def shear_stage(g, rect):
                band = band_pool.tile([32, B32COLS], mybir.dt.bfloat16, tag="band")
                rp = rect.ap[0][0]
                bp = band.ap[0][0]
                # q=0: src-side diagonal (base partition 0, exact runs)
                # The 4 shear DMAs ride 4 DIFFERENT engines so their DGE
                # stages (SWDGE prep ~1us / HWDGE ~0.6us) overlap instead of
                # serializing on one queue - they gate the PE transposes.
                nc.sync.dma_start(
                    out=AP(band.tensor, band.offset, [[bp, 32], [RUN, 3], [1, RUN]]),
                    in_=AP(rect.tensor, rect.offset, [[rp + HG, 32], [176 * HG, 3], [1, RUN]]))
                # q=1..3: plain 80-j window from partitions 32q.. shifted onto
                # the dst diagonal (stride pitch-HG writes partition p at col
                # DOFF - p*HG, so the k-diagonal sits at DOFF + k*HG).
                for q, eng in ((1, nc.scalar), (2, nc.gpsimd), (3, nc.sync)):
                    nci = 3 if q == 1 else 2
                    cb = next(c for (qq, cc, c, _) in _STRIPS if qq == q and cc == 0)
                    eng.dma_start(
                        out=AP(band.tensor, band.offset + cb + DOFF,
                               [[bp - HG, 32], [WSTRIP, nci], [1, WIN * HG]]),
                        in_=AP(rect.tensor, rect.offset + 32 * q * rp + 32 * q * HG,
                               [[rp, 32], [176 * HG, nci], [1, WIN * HG]]))
                return band

            def back_unit(band, outg, m):
                bp = band.ap[0][0]
                bt = bt_pool.tile([112, W], mybir.dt.bfloat16, tag="bt")
                # per (strip, h parity): in [32, 48k] -> out [48, 32] at
                # partition base e*64 (partitions 48..64 unused gap);
                # out partition q = e*64 + k, col = x.
                for q, ci, cb, doff in _STRIPS:
                    xcol = ci * 128 + 32 * q
                    for e in range(2):
                        nc.tensor.transpose(
                            bt[e * 64 : e * 64 + D, xcol : xcol + 32],
                            AP(band.tensor, band.offset + cb + doff + 2 * m + e,
                               [[bp, 32], [HG, D]]),
                            ident[:32, :32],
                        )
                ceng = nc.vector.tensor_copy if m % 2 else nc.scalar.copy
                ceng(outg[:, m * W : (m + 1) * W], bt[:, :])

            def store_group(g, outg):
                # Two stores per group (one per h parity): dst runs are full
                # (k, h) rows of W elems (640B). Partition q = e*64 + k ->
                # out[k, h0+2m+e, x]. The final group stores in half-m pieces
                # so the drain tail only waits on the last quarter of copies.
                h0 = g * HG
                ogp = outg.ap[0][0]
                nmh = 2
                mhw = HG // 2 // nmh
                for mh in range(nmh):
                    for e in range(2):
                        src = AP(outg.tensor,
                                 outg.offset + e * 64 * ogp + mh * mhw * W,
                                 [[ogp, D], [W, mhw], [1, W]])
                        dst = AP(out.tensor,
                                 out.offset + (h0 + 2 * mh * mhw + e) * W,
                                 [[HW, D], [2 * W, mhw], [1, W]])
                        nc.scalar.dma_start(out=dst, in_=src)

            # Unit-interleaved software pipeline. Iteration g emits:
            #   shear(g-1) first (its DGE runs while front(g) computes),
            #   front units of g interleaved with back units of g-1 at a
            #   SKEW-unit offset (PE is in-order: the transposes slot into
            #   the eviction-paced gaps between matmul units instead of
            #   serializing after all of front(g)),
            #   loads(g+1), then stores(g-1) (stores hold the SP SEQ on the
            #   copies' completion, so they go last).
            SKEW = 8
            NU = HG // 2
            pending = load_group(0)
            nc.sync.dma_start(out=ident[:, :], in_=ident_in[:, :])
            prev = None
            for g in range(NG):
                l_t, r_t = pending
                rect = rect_pool.tile([C, HG * SLOT], mybir.dt.bfloat16, tag="rect")
                binfo = None
                if prev is not None:
                    pg, prect = prev
                    band = shear_stage(pg, prect)
                    outg = outg_pool.tile([112, NU * W], mybir.dt.bfloat16, tag="outg")
                    binfo = (pg, band, outg)
                for p in range(NU):
                    front_unit(g, p, l_t, r_t, rect)
                    if binfo is not None and p >= SKEW:
                        back_unit(binfo[1], binfo[2], p - SKEW)
                if g + 1 < NG:
                    pending = load_group(g + 1)
                if binfo is not None:
                    for m in range(NU - SKEW, NU):
                        back_unit(binfo[1], binfo[2], m)
                    store_group(binfo[0], binfo[2])
                prev = (g, rect)
            pg, prect = prev
            band = shear_stage(pg, prect)
            outg = outg_pool.tile([112, NU * W], mybir.dt.bfloat16, tag="outg")
            for m in range(NU):
                back_unit(band, outg, m)
            store_group(pg, outg)
    nc.compile()
    return nc


def _get_nc(_mode="sbuf"):
    if _mode not in _cache:
        _cache[_mode] = _build(_mode)
    return _cache[_mode]


def kernel(left_feature, right_feature):
    import os
    left_feature = np.asarray(left_feature, dtype=np.float32)
    right_feature = np.asarray(right_feature, dtype=np.float32)
    b, c, h, w = left_feature.shape
    assert (b, c, h, w) == (B, C, H, W)
    nc = _get_nc()
    ident = np.eye(128, dtype=np.float32).astype(ml_dtypes.bfloat16)
    lf = (left_feature * (1.0 / C)).astype(ml_dtypes.bfloat16)
    rf = right_feature.astype(ml_dtypes.bfloat16)
    in_maps = []
    for i in range(B):
        in_maps.append({
            "left": np.ascontiguousarray(lf[i].reshape(C, HW)),
            "right": np.ascontiguousarray(rf[i].reshape(C, HW)),
            "ident": ident,
        })
    trace = bool(os.environ.get("KERNEL_TRACE"))
    res = run_bass_kernel_spmd(nc, in_maps, core_ids=list(range(B)), trace=trace)
    if trace:
        print("HW exec time:", res.exec_time_ns, "ns")
    outs = []
    for i in range(B):
        rev = res.results[i]["out"].astype(np.float32).reshape(D, H, W)
        outs.append(rev[::-1])  # device wrote k = 47 - i
    return np.stack(outs, axis=0)


if __name__ == "__main__":
    rng = np.random.default_rng(0)
    lf = rng.standard_normal((B, C, H, W), dtype=np.float32)
    rf = rng.standard_normal((B, C, H, W), dtype=np.float32)
    got = kernel(lf, rf)
    for (bb, i, hh, xx) in [(0, 0, 0, 0), (0, 5, 10, 100), (1, 47, 95, 319), (2, 47, 3, 10), (3, 20, 50, 10)]:
        want = float(np.dot(lf[bb, :, hh, xx], rf[bb, :, hh, xx - i]) / C) if xx >= i else 0.0
        print((bb, i, hh, xx), "got", got[bb, i, hh, xx], "want", want)


# revision 54
# speedup vs baseline: 1.1469x; 1.1469x over previous
"""Correlation cost volume kernel for Trainium2 (8 NeuronCores, data-parallel over batch).

cost[b, i, h, x] = mean_c left[b,c,h,x] * right[b,c,h,x-i],  i in [0,48), zero for x < i.

Per core (one batch element), all data bf16 (host converts; left pre-scaled 1/C):
  For each group of HG h rows:
    l_t [C, HG*320], r_t [C, HG*368] (47 left-pad zeros + data + 1 zero col).
    Per h row, 3 matmuls G[a, j] = sum_c l[c, X0+a] r_pad[c, X0+j] into psum
    bank slots (A[128x176] B[128x176] C[64x112] packed per 512-col fp32 bank,
    2 h rows per [128,1024] tile).
    Engine eviction (DVE/Act alternating, one [128, 2x464] op per 2h) writes
    the h-INTERLEAVED rect: rect[a, HG*(ci*176+j)+h] = G[a,h,ci,j], so each
    (a, ci) band row [j = a..a+48) x h is one contiguous 48*HG-elem run.
    Shear: direct SBUF->SBUF DMA. The DMA lowering only honors non-pitch
    partition strides for APs that start at partition 0 with <=32 partitions,
    so the diagonal is built per 32-partition block q:
      q=0: src-side diagonal (stride pitch+HG) -> exact 48*HG runs.
      q=1..3: plain 80-j src window (any base legal) + dst-side diagonal
      (stride pitch-HG, base 0) -> each strip holds the diagonal at col
      31*HG with windowed garbage around it (never read).
    band32 [32, 14848]: 3 exact strips (q0) + 7 windowed strips.
    PE transposes per (2h, parity e, strip): in [32, 48k] -> out [48, 32]
    at partition base e*64 (PSUM matmul outputs must start at partition
    0/32/64) into psum bt [112, 320], cols = x. Engine copy -> outg
    [112, 8*320]; two strided DMAs per group write the (k, h, x) output
    with full 320-elem (640B) runs (disparity reversed: device row
    k = cost[47-k]; host flips).
  Queue discipline: shear DMAs ride the Pool SWDGE path (non-blocking
  dispatch); SP gets l-load/out-store; Act gets only the right-load so its
  SEQ never stalls eviction/copy dispatches.
"""
import numpy as np
import ml_dtypes

import concourse.bacc as bacc
import concourse.mybir as mybir
import concourse.tile as tile
from concourse.ap import AP
from concourse.bass_utils import run_bass_kernel_spmd

B, C, H, W = 8, 128, 96, 320
D = 48
HG = 16         # h rows per group
NG = H // HG    # groups
RPAD = W + D    # 368: 47 left zeros, W data, 1 right zero
CHUNKS = [(0, 128, 0), (128, 128, 176), (256, 64, 352)]  # (X0, M, gcol)
SLOT = 464      # rect cols per h row (176+176+112)
HW = H * W
RUN = 48 * HG   # exact diagonal run per (a, ci)
WIN = 80        # windowed j span per 32-block
WSTRIP = 112 * HG   # windowed strip width (31 lead + 80 window + 1 pad)
DOFF = 31 * HG      # diagonal offset inside a windowed strip

# strips in band32 column order: (q, ci, colbase, diag_off)
_STRIPS = []
_col = 0
for _ci in range(3):
    _STRIPS.append((0, _ci, _col, 0))
    _col += RUN
for _q in range(1, 4):
    for _ci in range(3 if _q == 1 else 2):
        _STRIPS.append((_q, _ci, _col, DOFF))
        _col += WSTRIP
B32COLS = _col  # 14848

SHEAR_MODE = "sbuf"  # kept for test.py compat
_cache = {}


def _build(_mode="sbuf"):
    nc = bacc.Bacc("TRN2", target_bir_lowering=False, debug=False, num_devices=8)
    left = nc.dram_tensor("left", [C, HW], mybir.dt.bfloat16, kind="ExternalInput").ap()
    right = nc.dram_tensor("right", [C, HW], mybir.dt.bfloat16, kind="ExternalInput").ap()
    ident_in = nc.dram_tensor("ident", [128, 128], mybir.dt.bfloat16, kind="ExternalInput").ap()
    out = nc.dram_tensor("out", [D, HW], mybir.dt.bfloat16, kind="ExternalOutput").ap()

    with tile.TileContext(nc) as tc:
        with (
            tc.tile_pool(name="io", bufs=3) as io_pool,
            tc.tile_pool(name="rectp", bufs=3) as rect_pool,
            tc.tile_pool(name="bandp", bufs=2) as band_pool,
            tc.tile_pool(name="outp", bufs=4) as outg_pool,
            tc.tile_pool(name="const", bufs=1) as const_pool,
            tc.tile_pool(name="gps", bufs=3, space="PSUM") as g_pool,
            tc.tile_pool(name="bts", bufs=2, space="PSUM") as bt_pool,
        ):
            ident = const_pool.tile([128, 128], mybir.dt.bfloat16)

            def load_group(g):
                h0 = g * HG
                l_t = io_pool.tile([C, HG * W], mybir.dt.bfloat16, tag="lt")
                r_t = io_pool.tile([C, HG * RPAD], mybir.dt.bfloat16, tag="rt")
                rtp = r_t.ap[0][0]
                # pads: cols [0:47] and col 367 of each h row. Buffers rotate
                # with period 3 and loads only write data cols, so pads stay
                # zero after the first rotation.
                if g < 3:
                    nc.gpsimd.memset(
                        AP(r_t.tensor, r_t.offset, [[rtp, C], [RPAD, HG], [1, D - 1]]), 0.0)
                    nc.gpsimd.memset(
                        AP(r_t.tensor, r_t.offset + RPAD - 1, [[rtp, C], [RPAD, HG], [1, 1]]), 0.0)
                nc.sync.dma_start(out=l_t[:, :], in_=left[:, h0 * W : (h0 + HG) * W])
                r_dst = AP(r_t.tensor, r_t.offset + (D - 1), [[rtp, C], [RPAD, HG], [1, W]])
                nc.scalar.dma_start(out=r_dst, in_=right[:, h0 * W : (h0 + HG) * W])
                return l_t, r_t

            def front_unit(g, p, l_t, r_t, rect):
                rp = rect.ap[0][0]
                gt = g_pool.tile([128, 1024], mybir.dt.float32, tag="g")
                gp = gt.ap[0][0]
                for e in range(2):
                    hl = 2 * p + e
                    for X0, M, gcol in CHUNKS:
                        NW = M + D
                        nc.tensor.matmul(
                            gt[:M, 512 * e + gcol : 512 * e + gcol + NW],
                            l_t[:, hl * W + X0 : hl * W + X0 + M],
                            r_t[:, hl * RPAD + X0 : hl * RPAD + X0 + NW],
                            start=True, stop=True,
                        )
                eng = nc.vector if p % 2 == 0 else nc.scalar
                ev = eng.tensor_copy if p % 2 == 0 else eng.copy
                ev(
                    AP(rect.tensor, rect.offset + 2 * p,
                       [[rp, 128], [1, 2], [HG, SLOT]]),
                    AP(gt.tensor, gt.offset, [[gp, 128], [512, 2], [1, SLOT]]),
                )

            def shear_stage(g, rect):
                band = band_pool.tile([32, B32COLS], mybir.dt.bfloat16, tag="band")
                rp = rect.ap[0][0]
                bp = band.ap[0][0]
                # q=0: src-side diagonal (base partition 0, exact runs)
                # The 4 shear DMAs ride 4 DIFFERENT engines so their DGE
                # stages (SWDGE prep ~1us / HWDGE ~0.6us) overlap instead of
                # serializing on one queue - they gate the PE transposes.
                nc.sync.dma_start(
                    out=AP(band.tensor, band.offset, [[bp, 32], [RUN, 3], [1, RUN]]),
                    in_=AP(rect.tensor, rect.offset, [[rp + HG, 32], [176 * HG, 3], [1, RUN]]))
                # q=1..3: plain 80-j window from partitions 32q.. shifted onto
                # the dst diagonal (stride pitch-HG writes partition p at col
                # DOFF - p*HG, so the k-diagonal sits at DOFF + k*HG).
                for q, eng in ((1, nc.scalar), (2, nc.gpsimd), (3, nc.sync)):
                    nci = 3 if q == 1 else 2
                    cb = next(c for (qq, cc, c, _) in _STRIPS if qq == q and cc == 0)
                    eng.dma_start(
                        out=AP(band.tensor, band.offset + cb + DOFF,
                               [[bp - HG, 32], [WSTRIP, nci], [1, WIN * HG]]),
                        in_=AP(rect.tensor, rect.offset + 32 * q * rp + 32 * q * HG,
                               [[rp, 32], [176 * HG, nci], [1, WIN * HG]]))
                return band

            def back_unit(band, outg, m):
                bp = band.ap[0][0]
                bt = bt_pool.tile([112, W], mybir.dt.bfloat16, tag="bt")
                # per (strip, h parity): in [32, 48k] -> out [48, 32] at
                # partition base e*64 (partitions 48..64 unused gap);
                # out partition q = e*64 + k, col = x.
                for q, ci, cb, doff in _STRIPS:
                    xcol = ci * 128 + 32 * q
                    for e in range(2):
                        nc.tensor.transpose(
                            bt[e * 64 : e * 64 + D, xcol : xcol + 32],
                            AP(band.tensor, band.offset + cb + doff + 2 * m + e,
                               [[bp, 32], [HG, D]]),
                            ident[:32, :32],
                        )
                ceng = nc.vector.tensor_copy if m % 2 else nc.scalar.copy
                ceng(outg[:, m * W : (m + 1) * W], bt[:, :])

            def store_group(g, outg):
                # Two stores per group (one per h parity): dst runs are full
                # (k, h) rows of W elems (640B). Partition q = e*64 + k ->
                # out[k, h0+2m+e, x]. The final group stores in half-m pieces
                # so the drain tail only waits on the last quarter of copies.
                h0 = g * HG
                ogp = outg.ap[0][0]
                nmh = 2
                mhw = HG // 2 // nmh
                for mh in range(nmh):
                    for e in range(2):
                        src = AP(outg.tensor,
                                 outg.offset + e * 64 * ogp + mh * mhw * W,
                                 [[ogp, D], [W, mhw], [1, W]])
                        dst = AP(out.tensor,
                                 out.offset + (h0 + 2 * mh * mhw + e) * W,
                                 [[HW, D], [2 * W, mhw], [1, W]])
                        nc.sync.dma_start(out=dst, in_=src)

            # Unit-interleaved software pipeline. Iteration g emits:
            #   shear(g-1) first (its DGE runs while front(g) computes),
            #   front units of g interleaved with back units of g-1 at a
            #   SKEW-unit offset (PE is in-order: the transposes slot into
            #   the eviction-paced gaps between matmul units instead of
            #   serializing after all of front(g)),
            #   loads(g+1), then stores(g-1) (stores hold the SP SEQ on the
            #   copies' completion, so they go last).
            SKEW = 8
            NU = HG // 2
            pending = load_group(0)
            nc.sync.dma_start(out=ident[:, :], in_=ident_in[:, :])
            prev = None
            for g in range(NG):
                l_t, r_t = pending
                rect = rect_pool.tile([C, HG * SLOT], mybir.dt.bfloat16, tag="rect")
                binfo = None
                if prev is not None:
                    pg, prect = prev
                    band = shear_stage(pg, prect)
                    outg = outg_pool.tile([112, NU * W], mybir.dt.bfloat16, tag="outg")
                    binfo = (pg, band, outg)
                for p in range(NU):
                    front_unit(g, p, l_t, r_t, rect)
                    if binfo is not None and p >= SKEW:
                        back_unit(binfo[1], binfo[2], p - SKEW)
                if g + 1 < NG:
                    pending = load_group(g + 1)
                if binfo is not None:
                    for m in range(NU - SKEW, NU):
                        back_unit(binfo[1], binfo[2], m)
                    store_group(binfo[0], binfo[2])
                prev = (g, rect)
            pg, prect = prev
            band = shear_stage(pg, prect)
            outg = outg_pool.tile([112, NU * W], mybir.dt.bfloat16, tag="outg")
            for m in range(NU):
                back_unit(band, outg, m)
            store_group(pg, outg)
    nc.compile()
    return nc


def _get_nc(_mode="sbuf"):
    if _mode not in _cache:
        _cache[_mode] = _build(_mode)
    return _cache[_mode]


def kernel(left_feature, right_feature):
    import os
    left_feature = np.asarray(left_feature, dtype=np.float32)
    right_feature = np.asarray(right_feature, dtype=np.float32)
    b, c, h, w = left_feature.shape
    assert (b, c, h, w) == (B, C, H, W)
    nc = _get_nc()
    ident = np.eye(128, dtype=np.float32).astype(ml_dtypes.bfloat16)
    lf = (left_feature * (1.0 / C)).astype(ml_dtypes.bfloat16)
    rf = right_feature.astype(ml_dtypes.bfloat16)
    in_maps = []
    for i in range(B):
        in_maps.append({
            "left": np.ascontiguousarray(lf[i].reshape(C, HW)),
            "right": np.ascontiguousarray(rf[i].reshape(C, HW)),
            "ident": ident,
        })
    trace = bool(os.environ.get("KERNEL_TRACE"))
    res = run_bass_kernel_spmd(nc, in_maps, core_ids=list(range(B)), trace=trace)
    if trace:
        print("HW exec time:", res.exec_time_ns, "ns")
    outs = []
    for i in range(B):
        rev = res.results[i]["out"].astype(np.float32).reshape(D, H, W)
        outs.append(rev[::-1])  # device wrote k = 47 - i
    return np.stack(outs, axis=0)


if __name__ == "__main__":
    rng = np.random.default_rng(0)
    lf = rng.standard_normal((B, C, H, W), dtype=np.float32)
    rf = rng.standard_normal((B, C, H, W), dtype=np.float32)
    got = kernel(lf, rf)
    for (bb, i, hh, xx) in [(0, 0, 0, 0), (0, 5, 10, 100), (1, 47, 95, 319), (2, 47, 3, 10), (3, 20, 50, 10)]:
        want = float(np.dot(lf[bb, :, hh, xx], rf[bb, :, hh, xx - i]) / C) if xx >= i else 0.0
        print((bb, i, hh, xx), "got", got[bb, i, hh, xx], "want", want)
